# revision 1
# baseline (speedup 1.0000x reference)
"""DeltaNet Trainium2 kernel (nn_DeltaNet_41961830482331).

Full module: qkv = x @ w_attn; per-(head,dim-group) standardization (ddof=1);
DeltaNet recurrence  S_t = S_{t-1}(0.99 I - 0.01 k k^T) + k v^T, o_t = S_t q_t;
y = o @ w_proj; out = x + y.

Sharding: 8 cores = 4 batches x 2 head-groups (6 heads each). Each core runs
the full pipeline for its (batch, head-group); host sums the two partial
y-projections per batch (w_proj is row-split across the head-group pair).

Recurrence math (chunked, chunk n=128, gamma=0.99, beta=0.01):
substituting S_t = g^t Sh_t turns the decayed update into plain DeltaNet
  Sh_t = Sh_{t-1}(I - b' k k^T) + k nu_t^T,  b' = beta/g, nu_t = g^-t v_t,
  o_t = Sh_t qh_t, qh_t = g^t q_t.
Per chunk (K rows k_t, Vh rows nu_t, Qh rows qh_t, start state Sh0):
  G   = stril(K K^T)            (strict lower)
  Mh  = (I + b' G)^{-1} (b'(stril(K Vh^T) K + K Sh0^T))        [solve]
  O   = tril(Qh Vh^T) K - tril(Qh K^T) Mh + Qh Sh0^T
  Shn = Sh0 + K^T Vh - Mh^T K ;  next Sh0 = g^n Shn
The triangular solve uses a truncated Neumann series (8 terms, evaluated as
4 double-steps with N^2 precomputed); end-to-end rel err vs fp64 ~ 1e-3.
"""

import numpy as np

B, T, C = 4, 1024, 768
NH, HS = 12, 64
HPC = NH // 2            # heads per core
GAMMA, BETA = 0.99, 0.01
BP = BETA / GAMMA        # beta'
NC_ = 128                # chunk length n
NCH = T // NC_           # chunks
GN = GAMMA ** NC_        # gamma^n
NSOLVE = 4               # double-steps (= 8 Neumann terms)

_cache: dict = {}


def _build_program(debug_taps=False):
    import concourse.bass as bass
    import concourse.tile as tile
    from concourse import bacc, mybir

    f32 = mybir.dt.float32
    f32r = mybir.dt.float32r
    bf16 = mybir.dt.bfloat16
    Alu = mybir.AluOpType

    nc = bacc.Bacc()

    # ---- DRAM parameters (per-core data; SPMD: same names on all cores) ----
    xT = nc.dram_tensor("xT", [C, T], f32, kind="ExternalInput")          # x[b].T
    wA = nc.dram_tensor("wA", [C, 3 * HPC * HS], f32, kind="ExternalInput")
    wP = nc.dram_tensor("wP", [HPC * HS, C], f32, kind="ExternalInput")
    xres = nc.dram_tensor("xres", [T, C], bf16, kind="ExternalInput")     # x[b] or 0
    gvec = nc.dram_tensor("gvec", [128, 2], f32, kind="ExternalInput")    # g^(p+1), g^-(p+1)
    ident = nc.dram_tensor("ident", [128, 128], bf16, kind="ExternalInput")
    y = nc.dram_tensor("y", [T, C], f32, kind="ExternalOutput")
    dbg = {}
    if debug_taps:
        for nm, shp, dt_ in [("d_qkv", [128, 1152], bf16), ("d_kn", [128, 64], bf16),
                        ("d_vn", [128, 64], bf16), ("d_qn", [128, 64], bf16),
                        ("d_tsb", [64, 512], bf16), ("d_gsb", [128, 640], bf16),
                        ("d_rsb", [128, 384], bf16), ("d_r2", [128, 384], bf16),
                        ("d_mneg", [128, 384], bf16), ("d_st", [64, 384], bf16),
                        ("d_mu", [128, 18], f32), ("d_rstd", [128, 18], f32),
                        ("d_outT", [128, 128], f32)]:
            dbg[nm] = nc.dram_tensor(nm, shp, dt_, kind="ExternalOutput")

    W3 = 3 * HPC * HS    # 1152
    KT = C // 128        # 6 contraction tiles for qkv proj
    KP = HPC * HS // 128 # 3 contraction tiles for out proj

    with tile.TileContext(nc) as tc:
        with (
            tc.tile_pool(name="persist", bufs=1) as persist,
            tc.tile_pool(name="qkvp", bufs=2) as qkvp,
            tc.tile_pool(name="statp", bufs=3) as statp,
            tc.tile_pool(name="natp", bufs=2) as natp,
            tc.tile_pool(name="tp", bufs=2) as tp,
            tc.tile_pool(name="gramp", bufs=2) as gramp,
            tc.tile_pool(name="solvep", bufs=2) as solvep,
            tc.tile_pool(name="stp", bufs=2) as stp,
            tc.tile_pool(name="yp", bufs=3) as yp,
            tc.tile_pool(name="ps_big", bufs=3, space="PSUM") as ps_big,
            tc.tile_pool(name="ps_small", bufs=3, space="PSUM") as ps_small,
            tc.tile_pool(name="ps_tr", bufs=2, space="PSUM") as ps_tr,
        ):
            # ---- load persistent operands ----
            xT_sb = persist.tile([128, KT, T], f32r)
            nc.gpsimd.dma_start(out=xT_sb, in_=xT.rearrange("(k p) t -> p k t", p=128).bitcast(f32r))
            wA_sb = persist.tile([128, KT, W3], f32r)
            nc.gpsimd.dma_start(out=wA_sb, in_=wA.rearrange("(k p) j -> p k j", p=128).bitcast(f32r))
            wP_sb = persist.tile([128, KP, C], f32r)
            nc.gpsimd.dma_start(out=wP_sb, in_=wP.rearrange("(k p) j -> p k j", p=128).bitcast(f32r))
            gv_sb = persist.tile([128, 2], f32)
            nc.gpsimd.dma_start(out=gv_sb, in_=gvec[:, :])
            id_sb = persist.tile([128, 128], bf16)
            nc.gpsimd.dma_start(out=id_sb, in_=ident[:, :])

            # outT accumulates O^T for the whole sequence: [ch=384, t=1024]
            outT_sb = persist.tile([128, KP, T], f32r)

            st_prev = stp.tile([128, 192], bf16)
            nc.vector.memset(st_prev, 0.0)

            for c in range(NCH):
                t0 = c * NC_
                # ======== qkv projection for rows t0:t0+128 (fp32r) ========
                qkv_sb = qkvp.tile([128, W3], bf16)
                for nblk in range(3):
                    pp = ps_big.tile([128, 384], f32, tag="psbig")
                    for k in range(KT):
                        nc.tensor.matmul(
                            pp[:, :],
                            lhsT=xT_sb[:, k, t0:t0 + 128],
                            rhs=wA_sb[:, k, 384 * nblk:384 * nblk + 384],
                            start=(k == 0), stop=(k == KT - 1),
                        )
                    nc.any.tensor_copy(qkv_sb[:, 384 * nblk:384 * nblk + 384], pp[:, :])

                if debug_taps and c == 0:
                    nc.gpsimd.dma_start(out=dbg["d_qkv"][:, :], in_=qkv_sb[:, :])
                qc = qkv_sb[:, :]
                # ======== stats: per 64-col group mean / unbiased rstd ========
                sq = statp.tile([128, W3], bf16, tag="sq")
                nc.scalar.square(sq[:, :], qc)
                mu = statp.tile([128, 18], f32, tag="mu")
                nc.vector.tensor_reduce(
                    mu[:, :], qc.rearrange("p (g d) -> p g d", d=64),
                    axis=mybir.AxisListType.X, op=Alu.add)
                nc.scalar.mul(mu[:, :], mu[:, :], 1.0 / 64.0)
                m2 = statp.tile([128, 18], f32, tag="m2")
                nc.vector.tensor_reduce(
                    m2[:, :], sq.rearrange("p (g d) -> p g d", d=64),
                    axis=mybir.AxisListType.X, op=Alu.add)
                nc.scalar.mul(m2[:, :], m2[:, :], 1.0 / 64.0)
                # var_unb = (m2 - mu^2) * 64/63 ; rstd = 1/sqrt(var_unb)
                v63 = statp.tile([128, 18], f32, tag="v63")
                nc.vector.tensor_mul(v63[:, :], mu[:, :], mu[:, :])
                nc.vector.scalar_tensor_tensor(
                    out=v63[:, :], in0=v63[:, :], scalar=-1.0, in1=m2[:, :],
                    op0=Alu.mult, op1=Alu.add)
                rstd = statp.tile([128, 18], f32, tag="rstd")
                nc.scalar.activation(rstd[:, :], v63[:, :],
                                     mybir.ActivationFunctionType.Sqrt,
                                     scale=64.0 / 63.0)
                nc.vector.reciprocal(rstd[:, :], rstd[:, :])
                # scaled rstd variants: q heads *= g^(p+1), v heads *= g^-(p+1)
                rsq = statp.tile([128, 6], f32, tag="rsq")
                nc.vector.tensor_mul(rsq[:, :], rstd[:, 0:6],
                                     gv_sb[:, 0:1].to_broadcast((128, 6)))
                rsv = statp.tile([128, 6], f32, tag="rsv")
                nc.vector.tensor_mul(rsv[:, :], rstd[:, 12:18],
                                     gv_sb[:, 1:2].to_broadcast((128, 6)))

                # ==== per-head chunk processing (parity-pair packed) ====
                # pair p holds heads (2p, 2p+1); odd heads' transposed
                # operands + state live at partitions 64:128.
                knp_l, vnp_l, qnp_l, tsb_l = [], [], [], []
                gram_l = []
                for p in range(HPC // 2):
                    knp = natp.tile([128, 128], bf16, tag=f"knp{p}")
                    vnp = natp.tile([128, 128], bf16, tag=f"vnp{p}")
                    qnp = natp.tile([128, 128], bf16, tag=f"qnp{p}")
                    for sub in range(2):
                        i = 2 * p + sub
                        sl = slice(64 * sub, 64 * sub + 64)
                        nc.gpsimd.tensor_scalar(
                            out=knp[:, sl], in0=qc[:, 384 + 64 * i:384 + 64 * i + 64],
                            scalar1=mu[:, 6 + i:7 + i], scalar2=rstd[:, 6 + i:7 + i],
                            op0=Alu.subtract, op1=Alu.mult)
                        nc.vector.tensor_scalar(
                            out=vnp[:, sl], in0=qc[:, 768 + 64 * i:768 + 64 * i + 64],
                            scalar1=mu[:, 12 + i:13 + i], scalar2=rsv[:, i:i + 1],
                            op0=Alu.subtract, op1=Alu.mult)
                        nc.vector.tensor_scalar(
                            out=qnp[:, sl], in0=qc[:, 64 * i:64 * i + 64],
                            scalar1=mu[:, i:i + 1], scalar2=rsq[:, i:i + 1],
                            op0=Alu.subtract, op1=Alu.mult)
                    knp_l.append(knp); vnp_l.append(vnp); qnp_l.append(qnp)

                    # transpose both heads at once: [K^T | Vh^T | Qh^T]
                    tps = ps_tr.tile([128, 384], bf16, tag="tps")
                    nc.tensor.transpose(tps[:, 0:128], knp[:, :], id_sb[:, :])
                    nc.tensor.transpose(tps[:, 128:256], vnp[:, :], id_sb[:, :])
                    nc.tensor.transpose(tps[:, 256:384], qnp[:, :], id_sb[:, :])
                    tsb = tp.tile([128, 384], bf16, tag=f"tsb{p}")
                    nc.any.tensor_copy(tsb[:, :], tps[:, :])
                    tsb_l.append(tsb)

                    for sub in range(2):
                        i = 2 * p + sub
                        po = 64 * sub
                        kt = tsb[po:po + 64, 0:128]
                        vt = tsb[po:po + 64, 128:256]
                        qt = tsb[po:po + 64, 256:384]
                        tpos = (po, 0)
                        # gram products: Graw*(-b') = (-b'K)K^T via evac scale;
                        # HvT = Vh K^T, FiT = Vh Qh^T, F2T = K Qh^T ([s,t])
                        gps = ps_big.tile([128, 512], f32, tag="psbig")
                        nc.tensor.matmul(gps[:, 0:128], lhsT=kt, rhs=kt,
                                         tile_position=tpos)
                        nc.tensor.matmul(gps[:, 384:512], lhsT=kt, rhs=qt,
                                         tile_position=tpos)
                        nc.tensor.matmul(gps[:, 128:256], lhsT=vt, rhs=kt,
                                         tile_position=tpos)
                        nc.tensor.matmul(gps[:, 256:384], lhsT=vt, rhs=qt,
                                         tile_position=tpos)
                        gsb = gramp.tile([128, 640], bf16, tag=f"gsb{i}")
                        nc.scalar.mul(gsb[:, 0:128], gps[:, 0:128], -BP)
                        nc.any.tensor_copy(gsb[:, 128:512], gps[:, 128:512])
                        # N_low_s: strict-lower-masked copy of unmasked N band
                        nc.gpsimd.affine_select(
                            out=gsb[:, 512:640], in_=gsb[:, 0:128],
                            compare_op=Alu.is_ge, fill=0.0, base=-1,
                            pattern=[[-1, 128]], channel_multiplier=1)
                        # then mask [N_up_s | HvT] strict upper (in place)
                        nc.gpsimd.affine_select(
                            out=gsb[:, 0:256].rearrange("p (b f) -> p b f", b=2),
                            in_=gsb[:, 0:256].rearrange("p (b f) -> p b f", b=2),
                            compare_op=Alu.is_ge, fill=0.0, base=-1,
                            pattern=[[0, 2], [1, 128]], channel_multiplier=-1)
                        # [FiT | F2T] inclusive upper (in place)
                        nc.gpsimd.affine_select(
                            out=gsb[:, 256:512].rearrange("p (b f) -> p b f", b=2),
                            in_=gsb[:, 256:512].rearrange("p (b f) -> p b f", b=2),
                            compare_op=Alu.is_ge, fill=0.0, base=0,
                            pattern=[[0, 2], [1, 128]], channel_multiplier=-1)
                        gram_l.append(gsb)
                        if debug_taps and c == 0 and i == 0:
                            nc.gpsimd.dma_start(out=dbg["d_tsb"][:, :], in_=tsb[0:64, :])
                            nc.gpsimd.dma_start(out=dbg["d_gsb"][:, :], in_=gsb[:, :])
                if debug_taps and c == 0:
                    nc.gpsimd.dma_start(out=dbg["d_kn"][:, :], in_=knp_l[0][:, 0:64])
                    nc.gpsimd.dma_start(out=dbg["d_vn"][:, :], in_=vnp_l[0][:, 0:64])
                    nc.gpsimd.dma_start(out=dbg["d_qn"][:, :], in_=qnp_l[0][:, 0:64])

                # ======== R = b' (HvT^T K + K Sh0^T)  [128, 6*64] ========
                rps = ps_small.tile([128, 384], f32, tag="pss")
                for i in range(HPC):
                    p, sub = divmod(i, 2)
                    po = 64 * sub
                    nc.tensor.matmul(rps[:, 64 * i:64 * i + 64],
                                     lhsT=gram_l[i][:, 128:256],
                                     rhs=knp_l[p][:, po:po + 64],
                                     start=True, stop=False)
                    nc.tensor.matmul(rps[:, 64 * i:64 * i + 64],
                                     lhsT=tsb_l[p][po:po + 64, 0:128],
                                     rhs=st_prev[po:po + 64, 64 * p:64 * p + 64],
                                     start=False, stop=True, tile_position=(po, 0))
                r_sb = solvep.tile([128, 384], bf16, tag="rsb")
                nc.scalar.mul(r_sb[:, :], rps[:, :], BP)
                if debug_taps and c == 0:
                    nc.gpsimd.dma_start(out=dbg["d_rsb"][:, :], in_=r_sb[:, :])

                # ======== solve: Mh' = (I + b'G)^{-1} R via 4 double-steps ====
                # N2T[i] = N_low_s^T N_up_s = b'^2 (striu G)^2  (lhsT for steps)
                n2_l = []
                for half in range(2):
                    n2ps = ps_big.tile([128, 384], f32, tag="psbig")
                    for j in range(3):
                        i = 3 * half + j
                        nc.tensor.matmul(n2ps[:, 128 * j:128 * j + 128],
                                         lhsT=gram_l[i][:, 512:640],
                                         rhs=gram_l[i][:, 0:128])
                    n2sb = solvep.tile([128, 384], bf16, tag=f"n2{half}")
                    nc.any.tensor_copy(n2sb[:, :], n2ps[:, :])
                    n2_l.append(n2sb)

                # R2 = R - b' N_raw R  (NR via N_up_s = -b' striu G, then + R)
                zps = ps_small.tile([128, 384], f32, tag="pss")
                nc.tensor.matmul(zps[:, :], lhsT=id_sb[:, :], rhs=r_sb[:, :],
                                 start=True, stop=False, skip_group_check=True)
                for i in range(HPC):
                    nc.tensor.matmul(zps[:, 64 * i:64 * i + 64],
                                     lhsT=gram_l[i][:, 0:128],
                                     rhs=r_sb[:, 64 * i:64 * i + 64],
                                     start=False, stop=(i == HPC - 1),
                                     skip_group_check=True)
                r2_sb = solvep.tile([128, 384], bf16, tag="r2sb")
                nc.any.tensor_copy(r2_sb[:, :], zps[:, :])
                if debug_taps and c == 0:
                    nc.gpsimd.dma_start(out=dbg["d_r2"][:, :], in_=r2_sb[:, :])

                z_sb = r2_sb
                for it in range(NSOLVE):
                    zps2 = ps_small.tile([128, 384], f32, tag="pss")
                    nc.tensor.matmul(zps2[:, :], lhsT=id_sb[:, :], rhs=r2_sb[:, :],
                                     start=True, stop=False, skip_group_check=True)
                    for i in range(HPC):
                        nc.tensor.matmul(zps2[:, 64 * i:64 * i + 64],
                                         lhsT=n2_l[i // 3][:, 128 * (i % 3):128 * (i % 3) + 128],
                                         rhs=z_sb[:, 64 * i:64 * i + 64],
                                         start=False, stop=(i == HPC - 1),
                                         skip_group_check=True)
                    if it < NSOLVE - 1:
                        z_new = solvep.tile([128, 384], bf16, tag=f"z{it % 2}")
                        nc.any.tensor_copy(z_new[:, :], zps2[:, :])
                    else:
                        z_new = solvep.tile([128, 384], bf16, tag="mneg")
                        nc.scalar.mul(z_new[:, :], zps2[:, :], -1.0)  # Mneg = -Mh'
                    z_sb = z_new
                mneg = z_sb
                if debug_taps and c == 0:
                    nc.gpsimd.dma_start(out=dbg["d_mneg"][:, :], in_=mneg[:, :])

                # ======== O^T = K^T FiT + Mneg^T F2T + Sh0 Qh^T  ========
                for p in range(HPC // 2):
                    ops = ps_small.tile([128, 128], f32, tag="pss")
                    for sub in range(2):
                        i = 2 * p + sub
                        po = 64 * sub
                        sl = slice(po, po + 64)
                        nc.tensor.matmul(ops[sl, :], lhsT=knp_l[p][:, sl],
                                         rhs=gram_l[i][:, 256:384],
                                         start=True, stop=False, tile_position=(0, po))
                        nc.tensor.matmul(ops[sl, :], lhsT=mneg[:, 64 * i:64 * i + 64],
                                         rhs=gram_l[i][:, 384:512],
                                         start=False, stop=False, tile_position=(0, po))
                        nc.tensor.matmul(ops[sl, :],
                                         lhsT=st_prev[sl, 64 * p:64 * p + 64],
                                         rhs=tsb_l[p][sl, 256:384],
                                         start=False, stop=True, tile_position=(po, po))
                    nc.any.tensor_copy(outT_sb[:, p, t0:t0 + 128], ops[:, :])

                # ======== state update: Shn^T = Sh0^T + Vh^T K + K^T Mneg ====
                sps = ps_small.tile([128, 192], f32, tag="pss")
                for i in range(HPC):
                    p, sub = divmod(i, 2)
                    po = 64 * sub
                    psl = slice(po, po + 64)
                    fsl = slice(64 * p, 64 * p + 64)
                    dsl = slice(64 * p, 64 * p + 64)
                    nc.tensor.matmul(sps[psl, fsl], lhsT=vnp_l[p][:, psl],
                                     rhs=knp_l[p][:, psl],
                                     start=True, stop=False, tile_position=(0, po))
                    nc.tensor.matmul(sps[psl, fsl], lhsT=knp_l[p][:, psl],
                                     rhs=mneg[:, 64 * i:64 * i + 64],
                                     start=False, stop=False, tile_position=(0, po))
                    nc.tensor.matmul(sps[psl, fsl], lhsT=id_sb[psl, psl],
                                     rhs=st_prev[psl, fsl],
                                     start=False, stop=True, tile_position=(po, po))
                st_new = stp.tile([128, 192], bf16)
                nc.scalar.mul(st_new[:, :], sps[:, :], GN)   # *= g^n
                if debug_taps and c == 0:
                    nc.gpsimd.dma_start(out=dbg["d_st"][:, :], in_=st_new[0:64, :].rearrange("p (a b) -> p a b", a=3).rearrange("p a b -> p (a b)"))
                    nc.gpsimd.dma_start(out=dbg["d_mu"][:, :], in_=mu[:, :])
                    nc.gpsimd.dma_start(out=dbg["d_rstd"][:, :], in_=rstd[:, :])
                    nc.gpsimd.dma_start(out=dbg["d_outT"][:, :], in_=outT_sb[:, 0, 0:128].bitcast(f32))
                st_prev = st_new

                # ======== output projection rows t0:t0+128 (fp32r) ========
                xr_sb = yp.tile([128, C], bf16, tag="xr")
                nc.gpsimd.dma_start(out=xr_sb[:, :], in_=xres[t0:t0 + 128, :])
                y_sb = yp.tile([128, C], f32, tag="ysb")
                for nblk in range(2):
                    ypp = ps_big.tile([128, 384], f32, tag="psbig")
                    for k in range(KP):
                        nc.tensor.matmul(
                            ypp[:, :],
                            lhsT=outT_sb[:, k, t0:t0 + 128],
                            rhs=wP_sb[:, k, 384 * nblk:384 * nblk + 384],
                            start=(k == 0), stop=(k == KP - 1),
                        )
                    nc.vector.tensor_add(y_sb[:, 384 * nblk:384 * nblk + 384],
                                         ypp[:, :],
                                         xr_sb[:, 384 * nblk:384 * nblk + 384])
                nc.gpsimd.dma_start(out=y[t0:t0 + 128, :], in_=y_sb[:, :])

    nc.finalize()
    return nc


def _host_inputs(x, w_attn, w_proj):
    """Build the 8 per-core input maps."""
    in_maps = []
    gvec = np.zeros((128, 2), np.float32)
    p = np.arange(1, 129, dtype=np.float64)
    gvec[:, 0] = GAMMA ** p
    gvec[:, 1] = GAMMA ** (-p)
    ident = np.eye(128, dtype=np.float32)
    import ml_dtypes
    ident_bf = ident.astype(ml_dtypes.bfloat16)
    for core in range(8):
        b, hg = divmod(core, 2)
        h0 = hg * HPC
        cols = []
        for blk in range(3):   # q, k, v column blocks of w_attn
            cols.append(w_attn[:, blk * C + h0 * HS: blk * C + (h0 + HPC) * HS])
        wA_s = np.ascontiguousarray(np.concatenate(cols, axis=1))      # [768, 1152]
        wP_s = np.ascontiguousarray(w_proj[h0 * HS:(h0 + HPC) * HS])   # [384, 768]
        xb = np.ascontiguousarray(x[b])                                # [1024, 768]
        xres = xb.astype(ml_dtypes.bfloat16) if hg == 0 else \
            np.zeros((T, C), ml_dtypes.bfloat16)
        in_maps.append({
            "xT": np.ascontiguousarray(xb.T),
            "wA": wA_s,
            "wP": wP_s,
            "xres": xres,
            "gvec": gvec,
            "ident": ident_bf,
        })
    return in_maps


def kernel(x, w_attn, w_proj):
    from concourse.bass_utils import run_bass_kernel_spmd

    if "nc" not in _cache:
        _cache["nc"] = _build_program()
    nc = _cache["nc"]

    in_maps = _host_inputs(np.asarray(x), np.asarray(w_attn), np.asarray(w_proj))
    res = run_bass_kernel_spmd(nc, in_maps, core_ids=list(range(8)))
    out = np.empty((B, T, C), np.float32)
    for b in range(B):
        out[b] = res.results[2 * b]["y"] + res.results[2 * b + 1]["y"]
    return out



# revision 11
# speedup vs baseline: 1.8140x; 1.8140x over previous
"""DeltaNet Trainium2 kernel (nn_DeltaNet_41961830482331), v3.

Full module: qkv = x @ w_attn; per-(head,dim-group) standardization (ddof=1);
DeltaNet recurrence  S_t = S_{t-1}(0.99 I - 0.01 k k^T) + k v^T, o_t = S_t q_t;
y = o @ w_proj; out = x + y.

Sharding: 8 cores = 4 batches x 2 head-groups (6 heads each); host sums the two
partial y-projections per batch plus the residual x.

Chunked recurrence (n=128, b' = beta/gamma, st = Sh0^T, 9-term Neumann solve):
  G = K K^T; Gu = striu(G), Gl = stril(G); N = b' Gl
  Rraw = VKm^T K + K st (VKm = striu(Vh K^T)); R = b' Rraw
  n2 = b'^2 Gl^T Gu (=N2^T); n3 = -b' Gl^T n2 (=-N3^T)
  z0 = R - N R + N2 R; w1 = z0 + n3^T z0; M = z0 + n3^T w1
  O^T = K^T VQm - M^T KQm + st^T Qh^T;  st' = g^n (st + Vh^T K - K^T M)

Emission is software-pipelined: B-stages (state-dependent) of chunk c are
interleaved with A-stages of chunk c+1 so the PE queue never sits behind the
stats/normalize/mask chains. All matmul operands bf16; exact f32 scales are
folded into PSUM-evacuation ops; masks are 0/1 bf16 constants applied during
evacuation (vector/gpsimd split by head parity).
"""

import numpy as np

B, T, C = 4, 1024, 768
NH, HS = 12, 64
HPC = NH // 2            # heads per core
GAMMA, BETA = 0.99, 0.01
BP = BETA / GAMMA        # beta'
NC_ = 128                # chunk length n
NCH = T // NC_           # chunks
GN = GAMMA ** NC_        # gamma^n
NIT = 2                  # N^3 applications -> 3 + 3*NIT Neumann terms

_cache: dict = {}


def _build_program():
    import concourse.bass as bass
    import concourse.tile as tile
    from concourse import bacc, mybir

    f32 = mybir.dt.float32
    bf16 = mybir.dt.bfloat16
    Alu = mybir.AluOpType
    Act = mybir.ActivationFunctionType

    nc = bacc.Bacc()

    # ---- DRAM (per-core; SPMD same names on all cores) ----
    xT = nc.dram_tensor("xT", [128, NCH * 768], bf16, kind="ExternalInput")
    wA = nc.dram_tensor("wA", [128, 6 * 1152], bf16, kind="ExternalInput")
    wP = nc.dram_tensor("wP", [128, 3 * 768], bf16, kind="ExternalInput")
    gvec = nc.dram_tensor("gvec", [128, 2], f32, kind="ExternalInput")
    cid = nc.dram_tensor("cid", [128, 128], bf16, kind="ExternalInput")
    cmask = nc.dram_tensor("cmask", [128, 512], f32, kind="ExternalInput")
    y = nc.dram_tensor("y", [T, C], bf16, kind="ExternalOutput")

    xT3 = xT.rearrange("p (c j) -> p c j", c=NCH)
    wA3 = wA.rearrange("p (k j) -> p k j", k=6)

    with tile.TileContext(nc) as tc:
        with (
            tc.tile_pool(name="persist", bufs=1) as persist,
            tc.tile_pool(name="statp", bufs=3) as statp,
            tc.tile_pool(name="qsp", bufs=3) as qsp,
            tc.tile_pool(name="natp", bufs=3) as natp,
            tc.tile_pool(name="tp", bufs=3) as tp,
            tc.tile_pool(name="gramp", bufs=3) as gramp,
            tc.tile_pool(name="solvep", bufs=3) as solvep,
            tc.tile_pool(name="stp", bufs=2) as stp,
            tc.tile_pool(name="outp", bufs=3) as outp,
            tc.tile_pool(name="yp", bufs=2) as yp,
            tc.tile_pool(name="ps_q", bufs=2, space="PSUM") as ps_q,
            tc.tile_pool(name="ps_g", bufs=3, space="PSUM") as ps_g,
            tc.tile_pool(name="ps_nn", bufs=3, space="PSUM") as ps_nn,
        ):
            # ---- persistent loads (split across SP and Pool DMA rings) ----
            wA_sb = persist.tile([128, 6, 1152], bf16)
            xc_sb = []
            t_ = persist.tile([128, 768], bf16, tag="xc0", name="xc0")
            nc.gpsimd.dma_start(out=t_, in_=xT3[:, 0, :])
            xc_sb.append(t_)
            for k in range(6):
                eng = nc.sync if k % 2 == 0 else nc.gpsimd
                eng.dma_start(out=wA_sb[:, k, :], in_=wA3[:, k, :])
            id_sb = persist.tile([128, 128], bf16)
            nc.gpsimd.dma_start(out=id_sb, in_=cid[:, :])
            msk = persist.tile([128, 512], f32)
            nc.sync.dma_start(out=msk, in_=cmask[:, :])
            gv = persist.tile([128, 2], f32)
            nc.gpsimd.dma_start(out=gv, in_=gvec[:, :])
            for c in range(1, NCH):
                t_ = persist.tile([128, 768], bf16, tag=f"xc{c}", name=f"xc{c}")
                (nc.sync if c % 2 == 0 else nc.gpsimd).dma_start(out=t_, in_=xT3[:, c, :])
                xc_sb.append(t_)
            wP_sb = persist.tile([128, 3, 768], bf16)
            nc.gpsimd.dma_start(out=wP_sb, in_=wP.rearrange("p (k j) -> p k j", k=3))

            st_prev = stp.tile([128, 192], bf16, tag="st0")
            nc.vector.memset(st_prev, 0.0)

            ctx = [dict() for _ in range(NCH)]
            st_list = [st_prev]

            def stage_qkv(c):
                """qkv matmuls + quick evac; grouped stats; broadcast normalize."""
                X = ctx[c]
                qs = []
                bn = statp.tile([128, 18, 8], f32, tag="bn")
                for nblk in range(3):
                    pq = ps_q.tile([128, 384], f32, tag="q", name=f"pq{nblk}")
                    for k in range(6):
                        nc.tensor.matmul(
                            pq[:, :],
                            lhsT=xc_sb[c][:, 128 * k:128 * k + 128],
                            rhs=wA_sb[:, k, 384 * nblk:384 * nblk + 384],
                            start=(k == 0), stop=(k == 5),
                        )
                    q_ = qsp.tile([128, 384], bf16, tag=f"qs{nblk}", name=f"qs{nblk}")
                    nc.scalar.copy(q_, pq)
                    for g in range(6):
                        nc.vector.bn_stats(bn[:, 6 * nblk + g, 0:6],
                                           q_[:, 64 * g:64 * g + 64])
                    qs.append(q_)

                def col(j):
                    return bn[:, :, j:j + 1].rearrange("p g o -> p (g o)")

                cvs = statp.tile([128, 18], f32, tag="cvs")
                nc.vector.tensor_add(cvs, col(2), col(5))
                dmn = statp.tile([128, 18], f32, tag="dmn")
                nc.vector.tensor_sub(dmn, col(1), col(4))
                dsq = statp.tile([128, 18], f32, tag="dsq")
                nc.vector.tensor_mul(dsq, dmn, dmn)
                var = statp.tile([128, 18], f32, tag="var")
                nc.vector.scalar_tensor_tensor(
                    out=var, in0=dsq, scalar=16.0, in1=cvs,
                    op0=Alu.mult, op1=Alu.add)
                # mu = 2*mean; rstd = 0.5/sqrt(var_unb) so (2x - mu)*rstd is exact
                sd = statp.tile([128, 18], f32, tag="sd")
                nc.scalar.activation(sd, var, Act.Sqrt, scale=4.0 / 63.0)
                rstd = statp.tile([128, 18], f32, tag="rstd")
                nc.vector.reciprocal(rstd, sd)
                mu = statp.tile([128, 18], f32, tag="mu")
                nc.vector.tensor_add(mu, col(1), col(4))
                nc.vector.tensor_scalar(
                    out=rstd[:, 0:6], in0=rstd[:, 0:6], scalar1=gv[:, 0:1],
                    scalar2=None, op0=Alu.mult)
                nc.vector.tensor_scalar(
                    out=rstd[:, 12:18], in0=rstd[:, 12:18], scalar1=gv[:, 1:2],
                    scalar2=None, op0=Alu.mult)

                knp = [natp.tile([128, 128], bf16, tag=f"knp{p}", name=f"knp{p}") for p in range(3)]
                vnp = [natp.tile([128, 128], bf16, tag=f"vnp{p}", name=f"vnp{p}") for p in range(3)]
                qnp = [natp.tile([128, 128], bf16, tag=f"qnp{p}", name=f"qnp{p}") for p in range(3)]
                dst = [qnp, knp, vnp]
                for nblk in range(3):
                    mu_b = mu[:, 6 * nblk:6 * nblk + 6].rearrange(
                        "p g -> p g ()").to_broadcast((128, 6, 64))
                    r_b = rstd[:, 6 * nblk:6 * nblk + 6].rearrange(
                        "p g -> p g ()").to_broadcast((128, 6, 64))
                    tmp = statp.tile([128, 384], bf16, tag=f"tmp{nblk}")
                    nc.vector.scalar_tensor_tensor(
                        out=tmp.rearrange("p (g d) -> p g d", d=64),
                        in0=qs[nblk].rearrange("p (g d) -> p g d", d=64),
                        scalar=2.0, in1=mu_b, op0=Alu.mult, op1=Alu.subtract)
                    for p in range(3):
                        nc.vector.tensor_mul(
                            dst[nblk][p].rearrange("p (g d) -> p g d", d=64),
                            tmp[:, 128 * p:128 * p + 128].rearrange(
                                "p (g d) -> p g d", d=64),
                            r_b[:, 2 * p:2 * p + 2, :])
                X["knp"], X["vnp"], X["qnp"] = knp, vnp, qnp

            def stage_tr(c):
                """transposes -> tsb = [kt | qt | vt] per pair."""
                X = ctx[c]
                tsb = []
                for p in range(3):
                    tps = ps_g.tile([128, 384], bf16, tag="g", name="tps")
                    nc.tensor.transpose(tps[:, 0:128], X["knp"][p], id_sb)
                    nc.tensor.transpose(tps[:, 128:256], X["qnp"][p], id_sb)
                    nc.tensor.transpose(tps[:, 256:384], X["vnp"][p], id_sb)
                    t_ = tp.tile([128, 384], bf16, tag=f"tsb{p}", name=f"tsb{p}")
                    nc.scalar.copy(t_, tps)
                    tsb.append(t_)
                X["tsb"] = tsb

            def stage_gram(c):
                """grams + masked evac -> gsb = [Gu | VKm | KQm | VQm | Gl]."""
                X = ctx[c]
                gsb = []
                for i in range(6):
                    p, sub = divmod(i, 2)
                    po = 64 * sub
                    ts = X["tsb"][p]
                    kt = ts[po:po + 64, 0:128]
                    vt = ts[po:po + 64, 256:384]
                    rhs2 = ts[po:po + 64, 0:256]
                    gps = ps_g.tile([128, 512], f32, tag="g", name="gps")
                    nc.tensor.matmul(gps[:, 0:256], lhsT=kt, rhs=rhs2)
                    nc.tensor.matmul(gps[:, 256:512], lhsT=vt, rhs=rhs2)
                    g_ = gramp.tile([128, 640], bf16, tag=f"gsb{i}", name=f"gsb{i}")
                    perm = gps.rearrange("p (i j f) -> p j i f", i=2, j=2)
                    nc.vector.tensor_mul(
                        g_[:, 0:512].rearrange("p (j i f) -> p j i f", j=2, i=2),
                        perm,
                        msk[:, 0:512].rearrange("p (j i f) -> p j i f", j=2, i=2))
                    nc.gpsimd.affine_select(
                        out=g_[:, 512:640], in_=g_[:, 0:128],
                        compare_op=Alu.is_ge, fill=0.0, base=-1,
                        pattern=[[-1, 128]], channel_multiplier=1)
                    nc.gpsimd.affine_select(
                        out=g_[:, 0:128], in_=g_[:, 0:128],
                        compare_op=Alu.is_ge, fill=0.0, base=-1,
                        pattern=[[1, 128]], channel_multiplier=-1)
                    gsb.append(g_)
                X["gsb"] = gsb

            def stage_nn(c):
                """n2 = b'^2 Gl^T Gu ; n3 = -b' Gl^T n2 (both bf16)."""
                X = ctx[c]
                n2, n3 = [], []
                for half in range(2):
                    pp = ps_nn.tile([128, 384], f32, tag="nn", name="n2p")
                    for j in range(3):
                        g_ = X["gsb"][3 * half + j]
                        nc.tensor.matmul(pp[:, 128 * j:128 * j + 128],
                                         lhsT=g_[:, 512:640], rhs=g_[:, 0:128])
                    t_ = solvep.tile([128, 384], bf16, tag=f"n2{half}", name=f"n2{half}")
                    nc.scalar.copy(t_, pp)
                    n2.append(t_)
                for half in range(2):
                    pp = ps_nn.tile([128, 384], f32, tag="nn", name="n3p")
                    for j in range(3):
                        g_ = X["gsb"][3 * half + j]
                        nc.tensor.matmul(pp[:, 128 * j:128 * j + 128],
                                         lhsT=g_[:, 512:640],
                                         rhs=n2[half][:, 128 * j:128 * j + 128])
                    t_ = solvep.tile([128, 384], bf16, tag=f"n3{half}", name=f"n3{half}")
                    nc.scalar.copy(t_, pp)
                    n3.append(t_)
                X["n2"], X["n3"] = n2, n3

            def stage_r(c):
                """R = b'(VKm^T K + K st); also rbp = -b'^2 Rraw."""
                X = ctx[c]
                rp = ps_nn.tile([128, 384], f32, tag="nn", name="rp")
                for i in range(6):
                    p, sub = divmod(i, 2)
                    po = 64 * sub
                    sl = slice(64 * i, 64 * i + 64)
                    nc.tensor.matmul(rp[:, sl], lhsT=X["gsb"][i][:, 128:256],
                                     rhs=X["knp"][p][:, po:po + 64],
                                     start=True, stop=False)
                    nc.tensor.matmul(rp[:, sl],
                                     lhsT=X["tsb"][p][po:po + 64, 0:128],
                                     rhs=st_list[c][po:po + 64, 64 * p:64 * p + 64],
                                     start=False, stop=True)
                r_ = solvep.tile([128, 384], bf16, tag="rsb", name="rsb")
                nc.scalar.mul(r_, rp, BP)
                X["r"] = r_

            def stage_z0(c):
                """z0 = R - N R + N2 R."""
                X = ctx[c]
                zp = ps_nn.tile([128, 384], f32, tag="nn", name="zp")
                for i in range(6):
                    sl = slice(64 * i, 64 * i + 64)
                    nc.tensor.matmul(zp[:, sl], lhsT=X["gsb"][i][:, 0:128],
                                     rhs=X["r"][:, sl], start=True, stop=False)
                    nc.tensor.matmul(zp[:, sl],
                                     lhsT=X["n2"][i // 3][:, 128 * (i % 3):128 * (i % 3) + 128],
                                     rhs=X["r"][:, sl], start=False, stop=True)
                z0 = solvep.tile([128, 384], bf16, tag="z0", name="z0")
                nc.vector.tensor_add(z0, zp, X["r"])
                X["z0"] = z0

            def stage_w(c, it):
                """w_{it+1} = z0 + n3^T w_it  (last iter emits mneg = -M)."""
                X = ctx[c]
                src = X["z0"] if it == 0 else X[f"w{it}"]
                wp_ = ps_nn.tile([128, 384], f32, tag="nn", name="wp")
                for i in range(6):
                    sl = slice(64 * i, 64 * i + 64)
                    nc.tensor.matmul(wp_[:, sl],
                                     lhsT=X["n3"][i // 3][:, 128 * (i % 3):128 * (i % 3) + 128],
                                     rhs=src[:, sl])
                if it < NIT - 1:
                    t_ = solvep.tile([128, 384], bf16, tag=f"w{it + 1}", name=f"w{it + 1}")
                    nc.vector.tensor_add(t_, wp_, X["z0"])
                    X[f"w{it + 1}"] = t_
                else:
                    t_ = solvep.tile([128, 384], bf16, tag="mneg", name="mneg")
                    nc.vector.scalar_tensor_tensor(
                        out=t_, in0=wp_, scalar=-1.0, in1=X["z0"],
                        op0=Alu.mult, op1=Alu.subtract)
                    X["mneg"] = t_

            def stage_ost(c):
                """O^T -> outT tile ; state update -> st_list[c+1]."""
                X = ctx[c]
                ops = ps_nn.tile([128, 384], f32, tag="nn", name="ops")
                for i in range(6):
                    p, sub = divmod(i, 2)
                    po = 64 * sub
                    sl = slice(po, po + 64)
                    osl = slice(128 * p, 128 * p + 128)
                    nc.tensor.matmul(ops[sl, osl], lhsT=X["knp"][p][:, sl],
                                     rhs=X["gsb"][i][:, 384:512],
                                     start=True, stop=False)
                    nc.tensor.matmul(ops[sl, osl],
                                     lhsT=X["mneg"][:, 64 * i:64 * i + 64],
                                     rhs=X["gsb"][i][:, 256:384],
                                     start=False, stop=False)
                    nc.tensor.matmul(ops[sl, osl],
                                     lhsT=st_list[c][sl, 64 * p:64 * p + 64],
                                     rhs=X["tsb"][p][po:po + 64, 128:256],
                                     start=False, stop=True)
                ot = outp.tile([128, 384], bf16, tag="outT", name="outT")
                nc.scalar.copy(ot, ops)
                X["outT"] = ot

                sp = ps_nn.tile([128, 192], f32, tag="nn", name="sps")
                for i in range(6):
                    p, sub = divmod(i, 2)
                    po = 64 * sub
                    psl = slice(po, po + 64)
                    fsl = slice(64 * p, 64 * p + 64)
                    nc.tensor.matmul(sp[psl, fsl], lhsT=X["vnp"][p][:, psl],
                                     rhs=X["knp"][p][:, psl],
                                     start=True, stop=False)
                    nc.tensor.matmul(sp[psl, fsl], lhsT=X["knp"][p][:, psl],
                                     rhs=X["mneg"][:, 64 * i:64 * i + 64],
                                     start=False, stop=True)
                stg = stp.tile([128, 192], bf16, tag="stg")
                nc.scalar.mul(stg, st_list[c], GN)
                st_new = stp.tile([128, 192], bf16, tag=f"st{(c + 1) % 2}", name=f"stn{c}")
                nc.vector.scalar_tensor_tensor(
                    out=st_new, in0=sp, scalar=GN, in1=stg,
                    op0=Alu.mult, op1=Alu.add)
                st_list.append(st_new)

            def stage_yout(c):
                """y[t0:t0+128] = outT^T @ wP (bf16 out)."""
                X = ctx[c]
                t0 = NC_ * c
                y_sb = yp.tile([128, 768], bf16, tag="ysb")
                for nb in range(2):
                    ypp = ps_nn.tile([128, 384], f32, tag="nn", name="ypp")
                    for k in range(3):
                        nc.tensor.matmul(
                            ypp[:, :],
                            lhsT=X["outT"][:, 128 * k:128 * k + 128],
                            rhs=wP_sb[:, k, 384 * nb:384 * nb + 384],
                            start=(k == 0), stop=(k == 2),
                        )
                    nc.scalar.copy(y_sb[:, 384 * nb:384 * nb + 384], ypp)
                nc.sync.dma_start(out=y[t0:t0 + 128, :], in_=y_sb)

            # ---- software-pipelined emission ----
            stage_qkv(0)
            stage_tr(0)
            stage_gram(0)
            stage_nn(0)
            for c in range(NCH):
                stage_r(c)
                if c + 1 < NCH:
                    stage_qkv(c + 1)
                stage_z0(c)
                stage_w(c, 0)
                if c + 1 < NCH:
                    stage_tr(c + 1)
                stage_w(c, 1)
                if c + 1 < NCH:
                    stage_gram(c + 1)
                stage_ost(c)
                if c + 1 < NCH:
                    stage_nn(c + 1)
                stage_yout(c)

    nc.finalize()
    return nc


def _host_inputs(x, w_attn, w_proj):
    """Build the 8 per-core input maps (all heavy tensors bf16)."""
    import ml_dtypes
    bf = ml_dtypes.bfloat16
    in_maps = []
    gvec = np.zeros((128, 2), np.float32)
    p = np.arange(1, 129, dtype=np.float64)
    gvec[:, 0] = GAMMA ** p
    gvec[:, 1] = GAMMA ** (-p)
    ident = np.eye(128, dtype=np.float32).astype(bf)
    on = np.full((128, 128), -BP, np.float32)
    iu = np.triu(np.ones((128, 128), np.float32), 1)
    iui = np.triu(np.ones((128, 128), np.float32), 0)
    cmask = np.concatenate([on, iu, iui, iui], axis=1)
    for core in range(8):
        b, hg = divmod(core, 2)
        h0 = hg * HPC
        cols = []
        for blk in range(3):   # q, k, v column blocks of w_attn
            cols.append(w_attn[:, blk * C + h0 * HS: blk * C + (h0 + HPC) * HS])
        wA_s = np.concatenate(cols, axis=1).astype(bf)            # [768, 1152]
        wA_s = np.ascontiguousarray(
            wA_s.reshape(6, 128, 1152).transpose(1, 0, 2).reshape(128, 6 * 1152))
        wP_s = w_proj[h0 * HS:(h0 + HPC) * HS].astype(bf)         # [384, 768]
        wP_s = np.ascontiguousarray(
            wP_s.reshape(3, 128, 768).transpose(1, 0, 2).reshape(128, 3 * 768))
        xTb = x[b].T.astype(bf)                                   # [768, 1024]
        xTb = np.ascontiguousarray(
            xTb.reshape(6, 128, 8, 128).transpose(1, 2, 0, 3).reshape(128, 8 * 768))
        in_maps.append({
            "xT": xTb,
            "wA": wA_s,
            "wP": wP_s,
            "gvec": gvec,
            "cid": ident,
            "cmask": cmask,
        })
    return in_maps


def kernel(x, w_attn, w_proj):
    from concourse.bass_utils import run_bass_kernel_spmd

    if "nc" not in _cache:
        _cache["nc"] = _build_program()
    nc = _cache["nc"]

    x = np.asarray(x)
    in_maps = _host_inputs(x, np.asarray(w_attn), np.asarray(w_proj))
    res = run_bass_kernel_spmd(nc, in_maps, core_ids=list(range(8)))
    out = np.empty((B, T, C), np.float32)
    for b in range(B):
        out[b] = (x[b]
                  + res.results[2 * b]["y"].astype(np.float32)
                  + res.results[2 * b + 1]["y"].astype(np.float32))
    return out
